# revision 18
# baseline (speedup 1.0000x reference)
"""Causal multi-head attention on 8 TRN2 NeuronCores.

Sharding: 8 cores = 2 batches (data parallel) x 4 head-groups (tensor
parallel, 4 heads each). Each core computes Q/K/V projections for its 4
heads over its batch, per-head causal softmax attention, and a partial
output projection. The host sums the 4 partial outputs per batch.

Per-core dataflow (all matmuls bf16 with fp32 PSUM accumulation):
  x (f32, HBM) --dma-cast--> xb (bf16) --PE transpose--> xT [d, t]
  QT[hk, t] = Wq.T @ xT ; KT likewise ; V[t, hk] = xT.T @ Wv (+ biases)
  per head pair, per 512-wide q block, per 128-wide k tile:
    S^T[k, q] = KT.T @ QT              (PSUM)
    P^T = exp(S^T / 8)                 (ScalarE, PSUM->SBUF bf16)
    causal zero of diagonal tiles      (DVE affine_select)
    Z^T[dh, q] += [V | 1].T @ P^T      (row 64 = softmax denominator)
  Z normalized by 1/denominator, stored as zT [hk, t] bf16
  out[t, d] = zT.T @ Wo + bo           (partial; host reduces over cores)
"""

import sys

if "/opt/trn_rl_repo" not in sys.path:
    sys.path.insert(0, "/opt/trn_rl_repo")

import numpy as np

import concourse.bass as bass
import concourse.mybir as mybir
import concourse.tile as tile
from concourse import bacc
from concourse.bass_utils import run_bass_kernel_spmd
from concourse.masks import make_identity

# Problem shape (hardcoded per contract)
B = 2            # batches
S = 2048         # sequence length (tokens per batch)
D = 1024         # d_model
H = 16           # total heads
HPC = 4          # heads per core
DH = 64          # head dim
HK = HPC * DH    # 256 = per-core projection width
P = 128          # partitions
NT = S // P      # 16 token tiles
ND = D // P      # 8 d_model tiles
QW = 512         # q block width
NQB = S // QW    # 4 q blocks
SCALE = 1.0 / 8.0  # 1/sqrt(DH)

F32 = mybir.dt.float32
BF16 = mybir.dt.bfloat16


def build_program(debug=False, hwdump=False):
    dbg = {}
    nc = bacc.Bacc("TRN2")
    dump_d = {}
    if hwdump:
        for nm, shape, dt in [
            ("dump_xT", [P, ND * S], BF16),
            ("dump_qT", [P, 2 * S], BF16),
            ("dump_kT", [P, 2 * S], BF16),
            ("dump_v", [P, NT * HPC * (DH + 1)], BF16),
            ("dump_zT", [P, 2 * S], BF16),
            ("dump_s00", [P, 2 * QW], F32),
            ("dump_p00", [P, 2 * QW], BF16),
            ("dump_z00", [DH + 1, QW], F32),
            ("dump_rep00", [DH, QW], F32),
            ("dump_cmask", [P, 4 * QW], BF16),
        ]:
            dump_d[nm] = nc.dram_tensor(nm, shape, dt, kind="ExternalOutput")

    x_d = nc.dram_tensor("x", [S, D], F32, kind="ExternalInput")
    wq_d = nc.dram_tensor("wq", [D, HK], F32, kind="ExternalInput")
    wk_d = nc.dram_tensor("wk", [D, HK], F32, kind="ExternalInput")
    wv_d = nc.dram_tensor("wv", [D, HK], F32, kind="ExternalInput")
    wo_d = nc.dram_tensor("wo", [HK, D], F32, kind="ExternalInput")
    bq_d = nc.dram_tensor("bq", [1, HK], F32, kind="ExternalInput")
    bk_d = nc.dram_tensor("bk", [1, HK], F32, kind="ExternalInput")
    bv_d = nc.dram_tensor("bv", [1, HK], F32, kind="ExternalInput")
    bo_d = nc.dram_tensor("bo", [1, D], F32, kind="ExternalInput")
    out_d = nc.dram_tensor("out", [S, D], F32, kind="ExternalOutput")

    with tile.TileContext(nc) as tc:
        with (
            tc.tile_pool(name="const", bufs=1) as const,
            tc.tile_pool(name="res", bufs=1) as res,
            tc.tile_pool(name="xstage", bufs=3) as xstage,
            tc.tile_pool(name="pbuf", bufs=4) as pbuf,
            tc.tile_pool(name="drain", bufs=3) as drains,
            tc.tile_pool(name="small", bufs=2) as small,
            tc.tile_pool(name="dscratch", bufs=3, space="DRAM") as dpool,
            tc.tile_pool(name="spsum", bufs=2, space="PSUM") as spool,
            tc.tile_pool(name="zpsum", bufs=2, space="PSUM") as zpool,
            tc.tile_pool(name="opsum", bufs=2, space="PSUM") as opool,
        ):
            # ---- constants / weights ----
            ident = const.tile([P, P], BF16)
            make_identity(nc, ident)

            # Causal masks for the 4 diagonal k-tile offsets: mask[p, v, c] =
            # 1.0 where (qb*QW + c) >= (j*P + p) with v = j - 4*qb, else 0.
            cmask = const.tile([P, 4, QW], BF16)
            nc.gpsimd.memset(cmask, 1.0)
            nc.gpsimd.affine_select(
                out=cmask, in_=cmask,
                pattern=[[-P, 4], [1, QW]],
                compare_op=mybir.AluOpType.is_ge,
                fill=0.0,
                base=0,
                channel_multiplier=-1,
            )

            wq_sb = const.tile([P, ND, HK], BF16)
            wk_sb = const.tile([P, ND, HK], BF16)
            wv_sb = const.tile([P, ND, HK], BF16)
            wo_sb = const.tile([P, 2, D], BF16)
            nc.gpsimd.dma_start(out=wq_sb, in_=wq_d[:, :].rearrange("(n p) h -> p n h", p=P))
            nc.gpsimd.dma_start(out=wk_sb, in_=wk_d[:, :].rearrange("(n p) h -> p n h", p=P))
            nc.gpsimd.dma_start(out=wv_sb, in_=wv_d[:, :].rearrange("(n p) h -> p n h", p=P))
            nc.gpsimd.dma_start(out=wo_sb, in_=wo_d[:, :].rearrange("(n p) d -> p n d", p=P))

            bq_sb = const.tile([P, 2], F32)
            bk_sb = const.tile([P, 2], F32)
            for m in range(2):
                nc.gpsimd.dma_start(
                    out=bq_sb[:, m : m + 1],
                    in_=bq_d[0:1, m * P : (m + 1) * P].rearrange("a b -> b a"),
                )
                nc.gpsimd.dma_start(
                    out=bk_sb[:, m : m + 1],
                    in_=bk_d[0:1, m * P : (m + 1) * P].rearrange("a b -> b a"),
                )

            rep_bv = const.tile([P, HK], F32)
            bvap = bv_d[0, :]
            nc.gpsimd.dma_start(
                out=rep_bv,
                in_=bass.AP(tensor=bvap.tensor, offset=bvap.offset,
                            ap=[[0, P]] + [list(p) for p in bvap.ap]),
            )
            rep_bo = const.tile([P, D], F32)
            boap = bo_d[0, :]
            nc.gpsimd.dma_start(
                out=rep_bo,
                in_=bass.AP(tensor=boap.tensor, offset=boap.offset,
                            ap=[[0, P]] + [list(p) for p in boap.ap]),
            )

            # ---- resident tensors ----
            xT = res.tile([P, ND, S], BF16)      # x transposed [d, t]
            qT = res.tile([P, 2, S], BF16)       # Q^T [hk, t]
            kT = res.tile([P, 2, S], BF16)       # K^T [hk, t]
            v_sb = res.tile([P, NT, HPC, DH + 1], BF16)  # V (token-major), col 64 = 1.0
            zT = res.tile([P, 2, S], BF16)       # Z^T [hk, t], post-normalize

            nc.vector.memset(v_sb[:, :, :, DH : DH + 1], 1.0)

            dbg.update(xT=xT, qT=qT, kT=kT, v_sb=v_sb, zT=zT, cmask=cmask,
                       rep_bv=rep_bv, rep_bo=rep_bo, wq_sb=wq_sb, wo_sb=wo_sb,
                       bq_sb=bq_sb, ident=ident)

            # ---- stage A: load x (cast bf16) and transpose ----
            for tt in range(NT):
                xb = xstage.tile([P, D], BF16, tag="xb")
                nc.gpsimd.dma_start(out=xb, in_=x_d[tt * P : (tt + 1) * P, :])
                for dj in range(ND):
                    tp = opool.tile([P, P], BF16, tag="o")
                    nc.tensor.transpose(tp, xb[:, dj * P : (dj + 1) * P], ident)
                    nc.vector.tensor_copy(
                        out=xT[:, dj, tt * P : (tt + 1) * P], in_=tp
                    )

            # ---- stage B: projections ----
            for w_sb, b_sb, dst in ((wq_sb, bq_sb, qT), (wk_sb, bk_sb, kT)):
                for mt in range(2):
                    for qb in range(NQB):
                        ps = spool.tile([P, QW], F32, tag="s")
                        for dj in range(ND):
                            nc.tensor.matmul(
                                ps,
                                lhsT=w_sb[:, dj, mt * P : (mt + 1) * P],
                                rhs=xT[:, dj, qb * QW : (qb + 1) * QW],
                                start=(dj == 0),
                                stop=(dj == ND - 1),
                            )
                        nc.vector.tensor_scalar(
                            dst[:, mt, qb * QW : (qb + 1) * QW],
                            ps,
                            b_sb[:, mt : mt + 1],
                            None,
                            mybir.AluOpType.add,
                        )

            for tt in range(NT):
                ps = spool.tile([P, HK], F32, tag="s")
                for dj in range(ND):
                    nc.tensor.matmul(
                        ps,
                        lhsT=xT[:, dj, tt * P : (tt + 1) * P],
                        rhs=wv_sb[:, dj, :],
                        start=(dj == 0),
                        stop=(dj == ND - 1),
                    )
                nc.vector.tensor_add(
                    v_sb[:, tt, :, 0:DH],
                    ps.rearrange("p (h w) -> p h w", h=HPC),
                    rep_bv.rearrange("p (h w) -> p h w", h=HPC),
                )

            # ---- stage C: attention (head pairs for PE row-group packing) ----
            for m in range(2):
                for qb in range(NQB):
                    nkt = 4 * qb + 4
                    zps = [
                        zpool.tile([DH + 1, QW], F32, tag="z", name=f"zp{i}")
                        for i in range(2)
                    ]
                    for j in range(nkt):
                        sp = spool.tile([P, 2, QW], F32, tag="s")
                        for hi in range(2):
                            pb = hi * 64
                            nc.tensor.matmul(
                                sp[:, hi, :],
                                lhsT=kT[pb : pb + 64, m, j * P : (j + 1) * P],
                                rhs=qT[pb : pb + 64, m, qb * QW : (qb + 1) * QW],
                                start=True,
                                stop=True,
                            )
                        pt = pbuf.tile([P, 2, QW], BF16, tag="p")
                        nc.scalar.activation(
                            out=pt, in_=sp,
                            func=mybir.ActivationFunctionType.Exp,
                            scale=SCALE,
                        )
                        if hwdump and m == 0 and qb == 0 and j == 0:
                            sdump = drains.tile([P, 2, QW], F32, tag="sdump")
                            nc.vector.tensor_copy(out=sdump, in_=sp)
                            nc.sync.dma_start(
                                out=dump_d["dump_s00"][:, :].rearrange(
                                    "p (a b) -> p a b", a=2),
                                in_=sdump,
                            )
                        if j >= 4 * qb:  # diagonal: zero where k > q
                            mk = cmask[:, j - 4 * qb, :]
                            mk2 = bass.AP(
                                tensor=mk.tensor, offset=mk.offset,
                                ap=[list(mk.ap[0]), [0, 2], list(mk.ap[1])],
                            )
                            nc.vector.tensor_mul(pt, pt, mk2)
                        if hwdump and m == 0 and qb == 0 and j == 0:
                            nc.sync.dma_start(
                                out=dump_d["dump_p00"][:, :].rearrange(
                                    "p (a b) -> p a b", a=2),
                                in_=pt,
                            )
                        for hi in range(2):
                            h = 2 * m + hi
                            nc.tensor.matmul(
                                zps[hi],
                                lhsT=v_sb[:, j, h, 0 : DH + 1],
                                rhs=pt[:, hi, :],
                                start=(j == 0),
                                stop=(j == nkt - 1),
                            )
                    for hi in range(2):
                        ell = small.tile([DH + 1, QW], F32, tag="ell")
                        nc.vector.tensor_copy(
                            out=ell[DH : DH + 1, :], in_=zps[hi][DH : DH + 1, :]
                        )
                        nc.vector.reciprocal(ell[DH : DH + 1, :], ell[DH : DH + 1, :])
                        eld = dpool.tile([1, QW], F32, tag="eld")
                        nc.sync.dma_start(out=eld, in_=ell[DH : DH + 1, :])
                        rep = small.tile([DH, QW], F32, tag="rep")
                        e = eld[0, :]
                        nc.gpsimd.dma_start(
                            out=rep,
                            in_=bass.AP(tensor=e.tensor, offset=e.offset,
                                        ap=[[0, DH]] + [list(p) for p in e.ap]),
                        )
                        if hwdump and m == 0 and qb == 0 and hi == 0:
                            zdump = drains.tile([DH + 1, QW], F32, tag="zdump")
                            nc.vector.tensor_copy(out=zdump, in_=zps[hi])
                            nc.sync.dma_start(out=dump_d["dump_z00"][:, :], in_=zdump)
                            nc.sync.dma_start(out=dump_d["dump_rep00"][:, :], in_=rep)
                        if hi == 0:
                            nc.vector.tensor_mul(
                                zT[0:DH, m, qb * QW : (qb + 1) * QW],
                                zps[hi][0:DH, :],
                                rep,
                            )
                        else:
                            zs = drains.tile([DH, QW], BF16, tag="zstage")
                            nc.vector.tensor_mul(zs, zps[hi][0:DH, :], rep)
                            nc.sync.dma_start(
                                out=zT[64:128, m, qb * QW : (qb + 1) * QW], in_=zs
                            )

            # ---- stage D: output projection (partial) ----
            for tt in range(NT):
                for dc in range(2):
                    op = opool.tile([P, QW], F32, tag="o")
                    for kt2 in range(2):
                        nc.tensor.matmul(
                            op,
                            lhsT=zT[:, kt2, tt * P : (tt + 1) * P],
                            rhs=wo_sb[:, kt2, dc * QW : (dc + 1) * QW],
                            start=(kt2 == 0),
                            stop=(kt2 == 1),
                        )
                    ost = drains.tile([P, QW], F32, tag="ost")
                    nc.vector.tensor_add(ost, op, rep_bo[:, dc * QW : (dc + 1) * QW])
                    nc.sync.dma_start(
                        out=out_d[tt * P : (tt + 1) * P, dc * QW : (dc + 1) * QW],
                        in_=ost,
                    )

            if hwdump:
                nc.sync.dma_start(
                    out=dump_d["dump_xT"][:, :].rearrange("p (a b) -> p a b", a=ND),
                    in_=xT)
                nc.sync.dma_start(
                    out=dump_d["dump_qT"][:, :].rearrange("p (a b) -> p a b", a=2),
                    in_=qT)
                nc.sync.dma_start(
                    out=dump_d["dump_kT"][:, :].rearrange("p (a b) -> p a b", a=2),
                    in_=kT)
                nc.sync.dma_start(
                    out=dump_d["dump_v"][:, :].rearrange(
                        "p (a b c) -> p a b c", a=NT, b=HPC),
                    in_=v_sb)
                nc.sync.dma_start(
                    out=dump_d["dump_zT"][:, :].rearrange("p (a b) -> p a b", a=2),
                    in_=zT)
                nc.sync.dma_start(
                    out=dump_d["dump_cmask"][:, :].rearrange("p (a b) -> p a b", a=4),
                    in_=cmask)

    nc.finalize()
    if debug:
        return nc, dbg
    return nc


_NC_CACHE = None


def get_nc():
    global _NC_CACHE
    if _NC_CACHE is None:
        _NC_CACHE = build_program()
    return _NC_CACHE


def shard_inputs(x, W_Q, W_K, W_V, W_O, b_Q, b_K, b_V, b_O):
    f = lambda a: np.ascontiguousarray(np.asarray(a), dtype=np.float32)
    in_maps = []
    for core in range(8):
        b, g = divmod(core, 4)
        hs = slice(g * HPC, (g + 1) * HPC)
        in_maps.append({
            "x": f(np.asarray(x)[b]),
            "wq": f(np.asarray(W_Q)[hs].transpose(1, 0, 2).reshape(D, HK)),
            "wk": f(np.asarray(W_K)[hs].transpose(1, 0, 2).reshape(D, HK)),
            "wv": f(np.asarray(W_V)[hs].transpose(1, 0, 2).reshape(D, HK)),
            "wo": f(np.asarray(W_O)[hs].reshape(HK, D)),
            "bq": f(np.asarray(b_Q)[hs].reshape(1, HK)),
            "bk": f(np.asarray(b_K)[hs].reshape(1, HK)),
            "bv": f(np.asarray(b_V)[hs].reshape(1, HK)),
            "bo": f(np.asarray(b_O) if g == 0 else np.zeros_like(np.asarray(b_O))).reshape(1, D),
        })
    return in_maps


def combine_outputs(results):
    out = np.zeros((B, S, D), dtype=np.float32)
    for core in range(8):
        b = core // 4
        out[b] += results[core]["out"]
    return out


def kernel(**inputs):
    nc = get_nc()
    in_maps = shard_inputs(**inputs)
    res = run_bass_kernel_spmd(nc, in_maps, list(range(8)))
    return combine_outputs(res.results)


# revision 21
# speedup vs baseline: 1.4066x; 1.4066x over previous
"""Causal multi-head attention on 8 TRN2 NeuronCores.

Sharding: 8 cores = 2 batches (data parallel) x 4 head-groups (tensor
parallel, 4 heads each). Each core computes Q/K/V projections for its 4
heads over its batch, per-head causal softmax attention, and a partial
output projection. The host sums the 4 partial outputs per batch.

Per-core dataflow (all matmuls bf16 with fp32 PSUM accumulation):
  x (f32, HBM) --dma-cast--> xb (bf16) --PE transpose--> xT [d, t]
  QT[hk, t] = Wq.T @ xT ; KT likewise ; V[t, hk] = xT.T @ Wv (+ biases)
  per head pair, per 512-wide q block, per 128-wide k tile:
    S^T[k, q] = KT.T @ QT              (PSUM)
    P^T = exp(S^T / 8)                 (ScalarE, PSUM->SBUF bf16)
    causal zero of diagonal tiles      (DVE affine_select)
    Z^T[dh, q] += [V | 1].T @ P^T      (row 64 = softmax denominator)
  Z normalized by 1/denominator, stored as zT [hk, t] bf16
  out[t, d] = zT.T @ Wo + bo           (partial; host reduces over cores)
"""

import sys

if "/opt/trn_rl_repo" not in sys.path:
    sys.path.insert(0, "/opt/trn_rl_repo")

import numpy as np

import concourse.bass as bass
import concourse.mybir as mybir
import concourse.tile as tile
from concourse import bacc
from concourse.bass_utils import run_bass_kernel_spmd
from concourse.masks import make_identity

# Problem shape (hardcoded per contract)
B = 2            # batches
S = 2048         # sequence length (tokens per batch)
D = 1024         # d_model
H = 16           # total heads
HPC = 4          # heads per core
DH = 64          # head dim
HK = HPC * DH    # 256 = per-core projection width
P = 128          # partitions
NT = S // P      # 16 token tiles
ND = D // P      # 8 d_model tiles
QW = 512         # q block width
NQB = S // QW    # 4 q blocks
SCALE = 1.0 / 8.0  # 1/sqrt(DH)

F32 = mybir.dt.float32
BF16 = mybir.dt.bfloat16


def build_program(debug=False, hwdump=False):
    dbg = {}
    nc = bacc.Bacc("TRN2")
    dump_d = {}
    if hwdump:
        for nm, shape, dt in [
            ("dump_xT", [P, ND * S], BF16),
            ("dump_qT", [P, 2 * S], BF16),
            ("dump_kT", [P, 2 * S], BF16),
            ("dump_v", [P, NT * HPC * (DH + 1)], BF16),
            ("dump_zT", [P, 2 * S], BF16),
            ("dump_s00", [P, 2 * QW], F32),
            ("dump_p00", [P, 2 * QW], BF16),
            ("dump_z00", [DH + 1, QW], F32),
            ("dump_rep00", [DH, QW], F32),
            ("dump_cmask", [P, 4 * QW], BF16),
        ]:
            dump_d[nm] = nc.dram_tensor(nm, shape, dt, kind="ExternalOutput")

    x_d = nc.dram_tensor("x", [S, D], F32, kind="ExternalInput")
    wq_d = nc.dram_tensor("wq", [D, HK], F32, kind="ExternalInput")
    wk_d = nc.dram_tensor("wk", [D, HK], F32, kind="ExternalInput")
    wv_d = nc.dram_tensor("wv", [D, HK], F32, kind="ExternalInput")
    wo_d = nc.dram_tensor("wo", [HK, D], F32, kind="ExternalInput")
    bq_d = nc.dram_tensor("bq", [1, HK], F32, kind="ExternalInput")
    bk_d = nc.dram_tensor("bk", [1, HK], F32, kind="ExternalInput")
    bv_d = nc.dram_tensor("bv", [1, HK], F32, kind="ExternalInput")
    bo_d = nc.dram_tensor("bo", [1, D], F32, kind="ExternalInput")
    out_d = nc.dram_tensor("out", [S, D], F32, kind="ExternalOutput")

    with tile.TileContext(nc) as tc:
        with (
            tc.tile_pool(name="const", bufs=1) as const,
            tc.tile_pool(name="res", bufs=1) as res,
            tc.tile_pool(name="xstage", bufs=3) as xstage,
            tc.tile_pool(name="pbuf", bufs=4) as pbuf,
            tc.tile_pool(name="drain", bufs=3) as drains,
            tc.tile_pool(name="small", bufs=2) as small,
            tc.tile_pool(name="dscratch", bufs=3, space="DRAM") as dpool,
            tc.tile_pool(name="spsum", bufs=2, space="PSUM") as spool,
            tc.tile_pool(name="zpsum", bufs=2, space="PSUM") as zpool,
            tc.tile_pool(name="opsum", bufs=2, space="PSUM") as opool,
        ):
            # ---- constants / weights ----
            ident = const.tile([P, P], BF16)
            make_identity(nc, ident)

            # Causal masks for the 4 diagonal k-tile offsets, duplicated for
            # the 2 heads of a pair so the mask-mult uses a plain dense AP:
            # mask[p, v, hi, c] = 1.0 where (qb*QW + c) >= (j*P + p), v = j - 4*qb.
            cmask = const.tile([P, 4, 2, QW], BF16)
            nc.gpsimd.memset(cmask, 1.0)
            nc.gpsimd.affine_select(
                out=cmask, in_=cmask,
                pattern=[[-P, 4], [0, 2], [1, QW]],
                compare_op=mybir.AluOpType.is_ge,
                fill=0.0,
                base=0,
                channel_multiplier=-1,
            )

            wq_sb = const.tile([P, ND, HK], BF16)
            wk_sb = const.tile([P, ND, HK], BF16)
            wv_sb = const.tile([P, ND, HK], BF16)
            wo_sb = const.tile([P, 2, D], BF16)
            nc.gpsimd.dma_start(out=wq_sb, in_=wq_d[:, :].rearrange("(n p) h -> p n h", p=P))
            nc.gpsimd.dma_start(out=wk_sb, in_=wk_d[:, :].rearrange("(n p) h -> p n h", p=P))
            nc.gpsimd.dma_start(out=wv_sb, in_=wv_d[:, :].rearrange("(n p) h -> p n h", p=P))
            nc.gpsimd.dma_start(out=wo_sb, in_=wo_d[:, :].rearrange("(n p) d -> p n d", p=P))

            bq_sb = const.tile([P, 2], F32)
            bk_sb = const.tile([P, 2], F32)
            for m in range(2):
                nc.gpsimd.dma_start(
                    out=bq_sb[:, m : m + 1],
                    in_=bq_d[0:1, m * P : (m + 1) * P].rearrange("a b -> b a"),
                )
                nc.gpsimd.dma_start(
                    out=bk_sb[:, m : m + 1],
                    in_=bk_d[0:1, m * P : (m + 1) * P].rearrange("a b -> b a"),
                )

            rep_bv = const.tile([P, HK], F32)
            bvap = bv_d[0, :]
            nc.gpsimd.dma_start(
                out=rep_bv,
                in_=bass.AP(tensor=bvap.tensor, offset=bvap.offset,
                            ap=[[0, P]] + [list(p) for p in bvap.ap]),
            )
            rep_bo = const.tile([P, D], F32)
            boap = bo_d[0, :]
            nc.gpsimd.dma_start(
                out=rep_bo,
                in_=bass.AP(tensor=boap.tensor, offset=boap.offset,
                            ap=[[0, P]] + [list(p) for p in boap.ap]),
            )

            # ---- resident tensors ----
            xT = res.tile([P, ND, S], BF16)      # x transposed [d, t]
            qT = res.tile([P, 2, S], BF16)       # Q^T [hk, t]
            kT = res.tile([P, 2, S], BF16)       # K^T [hk, t]
            v_sb = res.tile([P, NT, HPC, DH + 1], BF16)  # V (token-major), col 64 = 1.0
            zT = res.tile([P, 2, S], BF16)       # Z^T [hk, t], post-normalize

            nc.vector.memset(v_sb[:, :, :, DH : DH + 1], 1.0)

            dbg.update(xT=xT, qT=qT, kT=kT, v_sb=v_sb, zT=zT, cmask=cmask,
                       rep_bv=rep_bv, rep_bo=rep_bo, wq_sb=wq_sb, wo_sb=wo_sb,
                       bq_sb=bq_sb, ident=ident)

            # ---- stage A: load x (cast bf16) and transpose ----
            for tt in range(NT):
                xb = xstage.tile([P, D], BF16, tag="xb")
                nc.gpsimd.dma_start(out=xb, in_=x_d[tt * P : (tt + 1) * P, :])
                for dg in range(2):  # 4 transposes share one psum tile/drain
                    tp = opool.tile([P, 4, P], BF16, tag="o")
                    for di in range(4):
                        dj = dg * 4 + di
                        nc.tensor.transpose(
                            tp[:, di, :], xb[:, dj * P : (dj + 1) * P], ident
                        )
                    nc.vector.tensor_copy(
                        out=xT[:, dg * 4 : (dg + 1) * 4, tt * P : (tt + 1) * P],
                        in_=tp,
                    )

            # ---- stage B: projections ----
            for w_sb, b_sb, dst in ((wq_sb, bq_sb, qT), (wk_sb, bk_sb, kT)):
                for mt in range(2):
                    for qb in range(NQB):
                        ps = spool.tile([P, QW], F32, tag="s")
                        for dj in range(ND):
                            nc.tensor.matmul(
                                ps,
                                lhsT=w_sb[:, dj, mt * P : (mt + 1) * P],
                                rhs=xT[:, dj, qb * QW : (qb + 1) * QW],
                                start=(dj == 0),
                                stop=(dj == ND - 1),
                            )
                        nc.vector.tensor_scalar(
                            dst[:, mt, qb * QW : (qb + 1) * QW],
                            ps,
                            b_sb[:, mt : mt + 1],
                            None,
                            mybir.AluOpType.add,
                        )

            for tt in range(NT):
                ps = spool.tile([P, HK], F32, tag="s")
                for dj in range(ND):
                    nc.tensor.matmul(
                        ps,
                        lhsT=xT[:, dj, tt * P : (tt + 1) * P],
                        rhs=wv_sb[:, dj, :],
                        start=(dj == 0),
                        stop=(dj == ND - 1),
                    )
                nc.vector.tensor_add(
                    v_sb[:, tt, :, 0:DH],
                    ps.rearrange("p (h w) -> p h w", h=HPC),
                    rep_bv.rearrange("p (h w) -> p h w", h=HPC),
                )

            # ---- stage C: attention (head pairs for PE row-group packing) ----
            def issue_scores(m, qb, j):
                sp = spool.tile([P, 2, QW], F32, tag="s", name=f"sp_{m}_{qb}_{j}")
                for hi in range(2):
                    pb = hi * 64
                    nc.tensor.matmul(
                        sp[:, hi, :],
                        lhsT=kT[pb : pb + 64, m, j * P : (j + 1) * P],
                        rhs=qT[pb : pb + 64, m, qb * QW : (qb + 1) * QW],
                        start=True,
                        stop=True,
                    )
                return sp

            for m in range(2):
                for qb in range(NQB):
                    nkt = 4 * qb + 4
                    zps = [
                        zpool.tile([DH + 1, QW], F32, tag="z", name=f"zp{i}")
                        for i in range(2)
                    ]
                    sp_cur = issue_scores(m, qb, 0)
                    for j in range(nkt):
                        sp_next = issue_scores(m, qb, j + 1) if j + 1 < nkt else None
                        pt = pbuf.tile([P, 2, QW], BF16, tag="p")
                        nc.scalar.activation(
                            out=pt, in_=sp_cur,
                            func=mybir.ActivationFunctionType.Exp,
                            scale=SCALE,
                        )
                        if j >= 4 * qb:  # diagonal: zero where k > q
                            nc.vector.tensor_mul(pt, pt, cmask[:, j - 4 * qb, :, :])
                        for hi in range(2):
                            h = 2 * m + hi
                            nc.tensor.matmul(
                                zps[hi],
                                lhsT=v_sb[:, j, h, 0 : DH + 1],
                                rhs=pt[:, hi, :],
                                start=(j == 0),
                                stop=(j == nkt - 1),
                            )
                        sp_cur = sp_next
                    for hi in range(2):
                        ell = small.tile([DH + 1, QW], F32, tag="ell")
                        nc.vector.reciprocal_approx_fast(
                            out=ell[DH : DH + 1, :], in_=zps[hi][DH : DH + 1, :]
                        )
                        eld = dpool.tile([1, QW], F32, tag="eld")
                        nc.sync.dma_start(out=eld, in_=ell[DH : DH + 1, :])
                        rep = small.tile([DH, QW], F32, tag="rep")
                        e = eld[0, :]
                        nc.gpsimd.dma_start(
                            out=rep,
                            in_=bass.AP(tensor=e.tensor, offset=e.offset,
                                        ap=[[0, DH]] + [list(p) for p in e.ap]),
                        )
                        if hi == 0:
                            nc.vector.tensor_mul(
                                zT[0:DH, m, qb * QW : (qb + 1) * QW],
                                zps[hi][0:DH, :],
                                rep,
                            )
                        else:
                            zs = drains.tile([DH, QW], BF16, tag="zstage")
                            nc.vector.tensor_mul(zs, zps[hi][0:DH, :], rep)
                            nc.sync.dma_start(
                                out=zT[64:128, m, qb * QW : (qb + 1) * QW], in_=zs
                            )

            # ---- stage D: output projection (partial) ----
            for tt in range(NT):
                for dc in range(2):
                    op = opool.tile([P, QW], F32, tag="o")
                    for kt2 in range(2):
                        nc.tensor.matmul(
                            op,
                            lhsT=zT[:, kt2, tt * P : (tt + 1) * P],
                            rhs=wo_sb[:, kt2, dc * QW : (dc + 1) * QW],
                            start=(kt2 == 0),
                            stop=(kt2 == 1),
                        )
                    ost = drains.tile([P, QW], F32, tag="ost")
                    nc.vector.tensor_add(ost, op, rep_bo[:, dc * QW : (dc + 1) * QW])
                    nc.sync.dma_start(
                        out=out_d[tt * P : (tt + 1) * P, dc * QW : (dc + 1) * QW],
                        in_=ost,
                    )

            if hwdump:
                nc.sync.dma_start(
                    out=dump_d["dump_xT"][:, :].rearrange("p (a b) -> p a b", a=ND),
                    in_=xT)
                nc.sync.dma_start(
                    out=dump_d["dump_qT"][:, :].rearrange("p (a b) -> p a b", a=2),
                    in_=qT)
                nc.sync.dma_start(
                    out=dump_d["dump_kT"][:, :].rearrange("p (a b) -> p a b", a=2),
                    in_=kT)
                nc.sync.dma_start(
                    out=dump_d["dump_v"][:, :].rearrange(
                        "p (a b c) -> p a b c", a=NT, b=HPC),
                    in_=v_sb)
                nc.sync.dma_start(
                    out=dump_d["dump_zT"][:, :].rearrange("p (a b) -> p a b", a=2),
                    in_=zT)
                nc.sync.dma_start(
                    out=dump_d["dump_cmask"][:, :].rearrange("p (a b) -> p a b", a=4),
                    in_=cmask)

    nc.finalize()
    if debug:
        return nc, dbg
    return nc


_NC_CACHE = None


def get_nc():
    global _NC_CACHE
    if _NC_CACHE is None:
        _NC_CACHE = build_program()
    return _NC_CACHE


def shard_inputs(x, W_Q, W_K, W_V, W_O, b_Q, b_K, b_V, b_O):
    f = lambda a: np.ascontiguousarray(np.asarray(a), dtype=np.float32)
    in_maps = []
    for core in range(8):
        b, g = divmod(core, 4)
        hs = slice(g * HPC, (g + 1) * HPC)
        in_maps.append({
            "x": f(np.asarray(x)[b]),
            "wq": f(np.asarray(W_Q)[hs].transpose(1, 0, 2).reshape(D, HK)),
            "wk": f(np.asarray(W_K)[hs].transpose(1, 0, 2).reshape(D, HK)),
            "wv": f(np.asarray(W_V)[hs].transpose(1, 0, 2).reshape(D, HK)),
            "wo": f(np.asarray(W_O)[hs].reshape(HK, D)),
            "bq": f(np.asarray(b_Q)[hs].reshape(1, HK)),
            "bk": f(np.asarray(b_K)[hs].reshape(1, HK)),
            "bv": f(np.asarray(b_V)[hs].reshape(1, HK)),
            "bo": f(np.asarray(b_O) if g == 0 else np.zeros_like(np.asarray(b_O))).reshape(1, D),
        })
    return in_maps


def combine_outputs(results):
    out = np.zeros((B, S, D), dtype=np.float32)
    for core in range(8):
        b = core // 4
        out[b] += results[core]["out"]
    return out


def kernel(**inputs):
    nc = get_nc()
    in_maps = shard_inputs(**inputs)
    res = run_bass_kernel_spmd(nc, in_maps, list(range(8)))
    return combine_outputs(res.results)


# revision 25
# speedup vs baseline: 1.4985x; 1.0653x over previous
"""Causal multi-head attention on 8 TRN2 NeuronCores.

Sharding: 8 cores = 2 batches (data parallel) x 4 head-groups (tensor
parallel, 4 heads each). Each core computes Q/K/V projections for its 4
heads over its batch, per-head causal softmax attention, and a partial
output projection. The host sums the 4 partial outputs per batch.

Per-core dataflow (all matmuls bf16 with fp32 PSUM accumulation):
  x (f32, HBM) --dma-cast--> xb (bf16) --PE transpose--> xT [d, t]
  QT[hk, t] = Wq.T @ xT ; KT likewise ; V[t, hk] = xT.T @ Wv (+ biases)
  per head pair, per 512-wide q block, per 128-wide k tile:
    S^T[k, q] = KT.T @ QT              (PSUM)
    P^T = exp(S^T / 8)                 (ScalarE, PSUM->SBUF bf16)
    causal zero of diagonal tiles      (DVE affine_select)
    Z^T[dh, q] += [V | 1].T @ P^T      (row 64 = softmax denominator)
  Z normalized by 1/denominator, stored as zT [hk, t] bf16
  out[t, d] = zT.T @ Wo + bo           (partial; host reduces over cores)
"""

import sys

if "/opt/trn_rl_repo" not in sys.path:
    sys.path.insert(0, "/opt/trn_rl_repo")

import numpy as np

import concourse.bass as bass
import concourse.mybir as mybir
import concourse.tile as tile
from concourse import bacc
from concourse.bass_utils import run_bass_kernel_spmd
from concourse.masks import make_identity

# Problem shape (hardcoded per contract)
B = 2            # batches
S = 2048         # sequence length (tokens per batch)
D = 1024         # d_model
H = 16           # total heads
HPC = 4          # heads per core
DH = 64          # head dim
HK = HPC * DH    # 256 = per-core projection width
P = 128          # partitions
NT = S // P      # 16 token tiles
ND = D // P      # 8 d_model tiles
QW = 512         # q block width
NQB = S // QW    # 4 q blocks
SCALE = 1.0 / 8.0  # 1/sqrt(DH)

F32 = mybir.dt.float32
BF16 = mybir.dt.bfloat16


def build_program(debug=False, hwdump=False):
    dbg = {}
    nc = bacc.Bacc("TRN2")
    dump_d = {}
    if hwdump:
        for nm, shape, dt in [
            ("dump_xT", [P, ND * S], BF16),
            ("dump_qT", [P, 2 * S], BF16),
            ("dump_kT", [P, 2 * S], BF16),
            ("dump_v", [P, NT * HPC * (DH + 1)], BF16),
            ("dump_zT", [P, 2 * S], BF16),
            ("dump_s00", [P, 2 * QW], F32),
            ("dump_p00", [P, 2 * QW], BF16),
            ("dump_z00", [DH + 1, QW], F32),
            ("dump_rep00", [DH, QW], F32),
            ("dump_cmask", [P, 4 * QW], BF16),
        ]:
            dump_d[nm] = nc.dram_tensor(nm, shape, dt, kind="ExternalOutput")

    x_d = nc.dram_tensor("x", [S, D], F32, kind="ExternalInput")
    wq_d = nc.dram_tensor("wq", [D, HK], F32, kind="ExternalInput")
    wk_d = nc.dram_tensor("wk", [D, HK], F32, kind="ExternalInput")
    wv_d = nc.dram_tensor("wv", [D, HK], F32, kind="ExternalInput")
    wo_d = nc.dram_tensor("wo", [HK, D], F32, kind="ExternalInput")
    bq_d = nc.dram_tensor("bq", [1, HK], F32, kind="ExternalInput")
    bk_d = nc.dram_tensor("bk", [1, HK], F32, kind="ExternalInput")
    bv_d = nc.dram_tensor("bv", [1, HK], F32, kind="ExternalInput")
    bo_d = nc.dram_tensor("bo", [1, D], F32, kind="ExternalInput")
    out_d = nc.dram_tensor("out", [S, D], F32, kind="ExternalOutput")

    with tile.TileContext(nc) as tc:
        with (
            tc.tile_pool(name="const", bufs=1) as const,
            tc.tile_pool(name="res", bufs=1) as res,
            tc.tile_pool(name="xstage", bufs=3) as xstage,
            tc.tile_pool(name="pbuf", bufs=4) as pbuf,
            tc.tile_pool(name="drain", bufs=3) as drains,
            tc.tile_pool(name="small", bufs=2) as small,
            tc.tile_pool(name="dscratch", bufs=3, space="DRAM") as dpool,
            tc.tile_pool(name="spsum", bufs=2, space="PSUM") as spool,
            tc.tile_pool(name="zopsum", bufs=4, space="PSUM") as zpool,
        ):
            # ---- constants / weights ----
            ident = const.tile([P, P], BF16)
            make_identity(nc, ident)

            # Causal masks for the 4 diagonal k-tile offsets, duplicated for
            # the 2 heads of a pair so the mask-mult uses a plain dense AP:
            # mask[p, v, hi, c] = 1.0 where (qb*QW + c) >= (j*P + p), v = j - 4*qb.
            cmask = const.tile([P, 4, 2, QW], BF16)
            nc.gpsimd.memset(cmask, 1.0)
            nc.gpsimd.affine_select(
                out=cmask, in_=cmask,
                pattern=[[-P, 4], [0, 2], [1, QW]],
                compare_op=mybir.AluOpType.is_ge,
                fill=0.0,
                base=0,
                channel_multiplier=-1,
            )

            wq_sb = const.tile([P, ND, HK], BF16)
            wk_sb = const.tile([P, ND, HK], BF16)
            wv_sb = const.tile([P, ND, HK], BF16)
            wo_sb = const.tile([P, 2, D], BF16)
            for w_dram, w_bf, rr in (
                (wq_d, wq_sb, "(n p) h -> p n h"),
                (wk_d, wk_sb, "(n p) h -> p n h"),
                (wv_d, wv_sb, "(n p) h -> p n h"),
                (wo_d, wo_sb, "(n p) d -> p n d"),
            ):
                wf = xstage.tile(list(w_bf.shape), F32, tag="wstage", name="wf")
                nc.sync.dma_start(out=wf, in_=w_dram[:, :].rearrange(rr, p=P))
                nc.scalar.copy(out=w_bf, in_=wf)

            bq_sb = const.tile([P, 2], F32)
            bk_sb = const.tile([P, 2], F32)
            for m in range(2):
                nc.gpsimd.dma_start(
                    out=bq_sb[:, m : m + 1],
                    in_=bq_d[0:1, m * P : (m + 1) * P].rearrange("a b -> b a"),
                )
                nc.gpsimd.dma_start(
                    out=bk_sb[:, m : m + 1],
                    in_=bk_d[0:1, m * P : (m + 1) * P].rearrange("a b -> b a"),
                )

            rep_bv = const.tile([P, HK], F32)
            bvap = bv_d[0, :]
            nc.gpsimd.dma_start(
                out=rep_bv,
                in_=bass.AP(tensor=bvap.tensor, offset=bvap.offset,
                            ap=[[0, P]] + [list(p) for p in bvap.ap]),
            )
            rep_bo = const.tile([P, D], F32)
            boap = bo_d[0, :]
            nc.gpsimd.dma_start(
                out=rep_bo,
                in_=bass.AP(tensor=boap.tensor, offset=boap.offset,
                            ap=[[0, P]] + [list(p) for p in boap.ap]),
            )

            # ---- resident tensors ----
            xT = res.tile([P, ND, S], BF16)      # x transposed [d, t]
            qT = res.tile([P, 2, S], BF16)       # Q^T [hk, t]
            kT = res.tile([P, 2, S], BF16)       # K^T [hk, t]
            v_sb = res.tile([P, NT, HPC, DH + 1], BF16)  # V (token-major), col 64 = 1.0
            zT = res.tile([P, 2, S], BF16)       # Z^T [hk, t], post-normalize

            nc.vector.memset(v_sb[:, :, :, DH : DH + 1], 1.0)

            dbg.update(xT=xT, qT=qT, kT=kT, v_sb=v_sb, zT=zT, cmask=cmask,
                       rep_bv=rep_bv, rep_bo=rep_bo, wq_sb=wq_sb, wo_sb=wo_sb,
                       bq_sb=bq_sb, ident=ident)

            # ---- stage A: load x (cast bf16) and transpose ----
            for tt in range(NT):
                xf = xstage.tile([P, D], F32, tag="xf")
                nc.sync.dma_start(out=xf, in_=x_d[tt * P : (tt + 1) * P, :])
                xb = xstage.tile([P, D], BF16, tag="xb")
                nc.scalar.copy(out=xb, in_=xf)
                for dg in range(2):  # 4 transposes share one psum tile/drain
                    tp = zpool.tile([P, 4, P], BF16, tag="zo")
                    for di in range(4):
                        dj = dg * 4 + di
                        nc.tensor.transpose(
                            tp[:, di, :], xb[:, dj * P : (dj + 1) * P], ident
                        )
                    nc.vector.tensor_copy(
                        out=xT[:, dg * 4 : (dg + 1) * 4, tt * P : (tt + 1) * P],
                        in_=tp,
                    )

            # ---- stage B: projections ----
            for w_sb, b_sb, dst in ((wq_sb, bq_sb, qT), (wk_sb, bk_sb, kT)):
                for mt in range(2):
                    for qb in range(NQB):
                        ps = spool.tile([P, QW], F32, tag="s")
                        for dj in range(ND):
                            nc.tensor.matmul(
                                ps,
                                lhsT=w_sb[:, dj, mt * P : (mt + 1) * P],
                                rhs=xT[:, dj, qb * QW : (qb + 1) * QW],
                                start=(dj == 0),
                                stop=(dj == ND - 1),
                            )
                        nc.vector.tensor_scalar(
                            dst[:, mt, qb * QW : (qb + 1) * QW],
                            ps,
                            b_sb[:, mt : mt + 1],
                            None,
                            mybir.AluOpType.add,
                        )

            for tt in range(NT):
                ps = spool.tile([P, HK], F32, tag="s")
                for dj in range(ND):
                    nc.tensor.matmul(
                        ps,
                        lhsT=xT[:, dj, tt * P : (tt + 1) * P],
                        rhs=wv_sb[:, dj, :],
                        start=(dj == 0),
                        stop=(dj == ND - 1),
                    )
                nc.vector.tensor_add(
                    v_sb[:, tt, :, 0:DH],
                    ps.rearrange("p (h w) -> p h w", h=HPC),
                    rep_bv.rearrange("p (h w) -> p h w", h=HPC),
                )

            # ---- stage C: attention (head pairs for PE row-group packing) ----
            def issue_scores(m, qb, j):
                sp = spool.tile([P, 2, QW], F32, tag="s", name=f"sp_{m}_{qb}_{j}")
                for hi in range(2):
                    pb = hi * 64
                    nc.tensor.matmul(
                        sp[:, hi, :],
                        lhsT=kT[pb : pb + 64, m, j * P : (j + 1) * P],
                        rhs=qT[pb : pb + 64, m, qb * QW : (qb + 1) * QW],
                        start=True,
                        stop=True,
                    )
                return sp

            for m in range(2):
                for qb in range(NQB):
                    nkt = 4 * qb + 4
                    zps = [
                        zpool.tile([DH + 1, QW], F32, tag="zo", name=f"zp{i}")
                        for i in range(2)
                    ]
                    sp_cur = issue_scores(m, qb, 0)
                    for j in range(nkt):
                        sp_next = issue_scores(m, qb, j + 1) if j + 1 < nkt else None
                        pt = pbuf.tile([P, 2, QW], BF16, tag="p")
                        nc.scalar.activation(
                            out=pt, in_=sp_cur,
                            func=mybir.ActivationFunctionType.Exp,
                            scale=SCALE,
                        )
                        if j >= 4 * qb:  # diagonal: zero where k > q
                            nc.vector.tensor_mul(pt, pt, cmask[:, j - 4 * qb, :, :])
                        for hi in range(2):
                            h = 2 * m + hi
                            nc.tensor.matmul(
                                zps[hi],
                                lhsT=v_sb[:, j, h, 0 : DH + 1],
                                rhs=pt[:, hi, :],
                                start=(j == 0),
                                stop=(j == nkt - 1),
                            )
                        sp_cur = sp_next
                    for hi in range(2):
                        ell = small.tile([DH + 1, QW], F32, tag="ell")
                        nc.vector.reciprocal_approx_fast(
                            out=ell[DH : DH + 1, :], in_=zps[hi][DH : DH + 1, :]
                        )
                        eld = dpool.tile([1, QW], F32, tag="eld")
                        nc.sync.dma_start(out=eld, in_=ell[DH : DH + 1, :])
                        rep = small.tile([DH, QW], F32, tag="rep")
                        e = eld[0, :]
                        nc.gpsimd.dma_start(
                            out=rep,
                            in_=bass.AP(tensor=e.tensor, offset=e.offset,
                                        ap=[[0, DH]] + [list(p) for p in e.ap]),
                        )
                        if hi == 0:
                            nc.vector.tensor_mul(
                                zT[0:DH, m, qb * QW : (qb + 1) * QW],
                                zps[hi][0:DH, :],
                                rep,
                            )
                        else:
                            zs = drains.tile([DH, QW], BF16, tag="zstage")
                            nc.vector.tensor_mul(zs, zps[hi][0:DH, :], rep)
                            nc.sync.dma_start(
                                out=zT[64:128, m, qb * QW : (qb + 1) * QW], in_=zs
                            )

                    # ---- output projection for this q block (needs zT of
                    # both head pairs, so only after the m == 1 pass) ----
                    if m == 1:
                        for tt in range(4 * qb, 4 * qb + 4):
                            for dc in range(2):
                                op = zpool.tile([P, QW], F32, tag="zo", name="op")
                                for kt2 in range(2):
                                    nc.tensor.matmul(
                                        op,
                                        lhsT=zT[:, kt2, tt * P : (tt + 1) * P],
                                        rhs=wo_sb[:, kt2, dc * QW : (dc + 1) * QW],
                                        start=(kt2 == 0),
                                        stop=(kt2 == 1),
                                    )
                                ost = drains.tile([P, QW], F32, tag="ost")
                                nc.vector.tensor_add(
                                    ost, op, rep_bo[:, dc * QW : (dc + 1) * QW]
                                )
                                nc.sync.dma_start(
                                    out=out_d[
                                        tt * P : (tt + 1) * P,
                                        dc * QW : (dc + 1) * QW,
                                    ],
                                    in_=ost,
                                )

            if hwdump:
                nc.sync.dma_start(
                    out=dump_d["dump_xT"][:, :].rearrange("p (a b) -> p a b", a=ND),
                    in_=xT)
                nc.sync.dma_start(
                    out=dump_d["dump_qT"][:, :].rearrange("p (a b) -> p a b", a=2),
                    in_=qT)
                nc.sync.dma_start(
                    out=dump_d["dump_kT"][:, :].rearrange("p (a b) -> p a b", a=2),
                    in_=kT)
                nc.sync.dma_start(
                    out=dump_d["dump_v"][:, :].rearrange(
                        "p (a b c) -> p a b c", a=NT, b=HPC),
                    in_=v_sb)
                nc.sync.dma_start(
                    out=dump_d["dump_zT"][:, :].rearrange("p (a b) -> p a b", a=2),
                    in_=zT)
                nc.sync.dma_start(
                    out=dump_d["dump_cmask"][:, :].rearrange("p (a b) -> p a b", a=4),
                    in_=cmask)

    nc.finalize()
    if debug:
        return nc, dbg
    return nc


_NC_CACHE = None


def get_nc():
    global _NC_CACHE
    if _NC_CACHE is None:
        _NC_CACHE = build_program()
    return _NC_CACHE


def shard_inputs(x, W_Q, W_K, W_V, W_O, b_Q, b_K, b_V, b_O):
    f = lambda a: np.ascontiguousarray(np.asarray(a), dtype=np.float32)
    in_maps = []
    for core in range(8):
        b, g = divmod(core, 4)
        hs = slice(g * HPC, (g + 1) * HPC)
        in_maps.append({
            "x": f(np.asarray(x)[b]),
            "wq": f(np.asarray(W_Q)[hs].transpose(1, 0, 2).reshape(D, HK)),
            "wk": f(np.asarray(W_K)[hs].transpose(1, 0, 2).reshape(D, HK)),
            "wv": f(np.asarray(W_V)[hs].transpose(1, 0, 2).reshape(D, HK)),
            "wo": f(np.asarray(W_O)[hs].reshape(HK, D)),
            "bq": f(np.asarray(b_Q)[hs].reshape(1, HK)),
            "bk": f(np.asarray(b_K)[hs].reshape(1, HK)),
            "bv": f(np.asarray(b_V)[hs].reshape(1, HK)),
            "bo": f(np.asarray(b_O) if g == 0 else np.zeros_like(np.asarray(b_O))).reshape(1, D),
        })
    return in_maps


def combine_outputs(results):
    out = np.zeros((B, S, D), dtype=np.float32)
    for core in range(8):
        b = core // 4
        out[b] += results[core]["out"]
    return out


def kernel(**inputs):
    nc = get_nc()
    in_maps = shard_inputs(**inputs)
    res = run_bass_kernel_spmd(nc, in_maps, list(range(8)))
    return combine_outputs(res.results)


# revision 27
# speedup vs baseline: 1.7108x; 1.1417x over previous
"""Causal multi-head attention on 8 TRN2 NeuronCores.

Sharding: 8 cores = 2 batches (data parallel) x 4 head-groups (tensor
parallel, 4 heads each). Each core computes Q/K/V projections for its 4
heads over its batch, per-head causal softmax attention, and a partial
output projection. The host sums the 4 partial outputs per batch.

Per-core dataflow (all matmuls bf16 with fp32 PSUM accumulation):
  x (f32, HBM) --dma-cast--> xb (bf16) --PE transpose--> xT [d, t]
  QT[hk, t] = Wq.T @ xT ; KT likewise ; V[t, hk] = xT.T @ Wv (+ biases)
  per head pair, per 512-wide q block, per 128-wide k tile:
    S^T[k, q] = KT.T @ QT              (PSUM)
    P^T = exp(S^T / 8)                 (ScalarE, PSUM->SBUF bf16)
    causal zero of diagonal tiles      (DVE affine_select)
    Z^T[dh, q] += [V | 1].T @ P^T      (row 64 = softmax denominator)
  Z normalized by 1/denominator, stored as zT [hk, t] bf16
  out[t, d] = zT.T @ Wo + bo           (partial; host reduces over cores)
"""

import sys

if "/opt/trn_rl_repo" not in sys.path:
    sys.path.insert(0, "/opt/trn_rl_repo")

import numpy as np

import concourse.bass as bass
import concourse.mybir as mybir
import concourse.tile as tile
from concourse import bacc
from concourse.bass_utils import run_bass_kernel_spmd
from concourse.masks import make_identity

# Problem shape (hardcoded per contract)
B = 2            # batches
S = 2048         # sequence length (tokens per batch)
D = 1024         # d_model
H = 16           # total heads
HPC = 4          # heads per core
DH = 64          # head dim
HK = HPC * DH    # 256 = per-core projection width
P = 128          # partitions
NT = S // P      # 16 token tiles
ND = D // P      # 8 d_model tiles
QW = 512         # q block width
NQB = S // QW    # 4 q blocks
SCALE = 1.0 / 8.0  # 1/sqrt(DH)

F32 = mybir.dt.float32
BF16 = mybir.dt.bfloat16


def build_program(debug=False, hwdump=False):
    dbg = {}
    nc = bacc.Bacc("TRN2")
    dump_d = {}
    if hwdump:
        for nm, shape, dt in [
            ("dump_xT", [P, ND * S], BF16),
            ("dump_qT", [P, 2 * S], BF16),
            ("dump_kT", [P, 2 * S], BF16),
            ("dump_v", [P, NT * HPC * (DH + 1)], BF16),
            ("dump_zT", [P, 2 * S], BF16),
            ("dump_s00", [P, 2 * QW], F32),
            ("dump_p00", [P, 2 * QW], BF16),
            ("dump_z00", [DH + 1, QW], F32),
            ("dump_rep00", [DH, QW], F32),
            ("dump_cmask", [P, 4 * QW], BF16),
        ]:
            dump_d[nm] = nc.dram_tensor(nm, shape, dt, kind="ExternalOutput")

    x_d = nc.dram_tensor("x", [S, D], F32, kind="ExternalInput")
    wq_d = nc.dram_tensor("wq", [D, HK], F32, kind="ExternalInput")
    wk_d = nc.dram_tensor("wk", [D, HK], F32, kind="ExternalInput")
    wv_d = nc.dram_tensor("wv", [D, HK], F32, kind="ExternalInput")
    wo_d = nc.dram_tensor("wo", [HK, D], F32, kind="ExternalInput")
    bq_d = nc.dram_tensor("bq", [1, HK], F32, kind="ExternalInput")
    bk_d = nc.dram_tensor("bk", [1, HK], F32, kind="ExternalInput")
    bv_d = nc.dram_tensor("bv", [1, HK], F32, kind="ExternalInput")
    bo_d = nc.dram_tensor("bo", [1, D], F32, kind="ExternalInput")
    out_d = nc.dram_tensor("out", [S, D], F32, kind="ExternalOutput")

    with tile.TileContext(nc) as tc:
        with (
            tc.tile_pool(name="const", bufs=1) as const,
            tc.tile_pool(name="res", bufs=1) as res,
            tc.tile_pool(name="xstage", bufs=3) as xstage,
            tc.tile_pool(name="pbuf", bufs=4) as pbuf,
            tc.tile_pool(name="drain", bufs=3) as drains,
            tc.tile_pool(name="small", bufs=2) as small,
            tc.tile_pool(name="dscratch", bufs=3, space="DRAM") as dpool,
            tc.tile_pool(name="spsum", bufs=2, space="PSUM") as spool,
            tc.tile_pool(name="zopsum", bufs=4, space="PSUM") as zpool,
        ):
            # ---- constants / weights ----
            ident = const.tile([P, P], BF16)
            make_identity(nc, ident)

            wq_sb = const.tile([P, ND, HK], BF16)
            wk_sb = const.tile([P, ND, HK], BF16)
            wv_sb = const.tile([P, ND, HK], BF16)
            wo_sb = const.tile([P, 2, D], BF16)
            nc.gpsimd.dma_start(out=wv_sb, in_=wv_d[:, :].rearrange("(n p) h -> p n h", p=P))
            nc.gpsimd.dma_start(out=wq_sb, in_=wq_d[:, :].rearrange("(n p) h -> p n h", p=P))
            nc.gpsimd.dma_start(out=wk_sb, in_=wk_d[:, :].rearrange("(n p) h -> p n h", p=P))
            nc.gpsimd.dma_start(out=wo_sb, in_=wo_d[:, :].rearrange("(n p) d -> p n d", p=P))

            bq_sb = const.tile([P, 2], F32)
            bk_sb = const.tile([P, 2], F32)
            for m in range(2):
                nc.gpsimd.dma_start(
                    out=bq_sb[:, m : m + 1],
                    in_=bq_d[0:1, m * P : (m + 1) * P].rearrange("a b -> b a"),
                )
                nc.gpsimd.dma_start(
                    out=bk_sb[:, m : m + 1],
                    in_=bk_d[0:1, m * P : (m + 1) * P].rearrange("a b -> b a"),
                )

            rep_bv = const.tile([P, HK], F32)
            bvap = bv_d[0, :]
            nc.gpsimd.dma_start(
                out=rep_bv,
                in_=bass.AP(tensor=bvap.tensor, offset=bvap.offset,
                            ap=[[0, P]] + [list(p) for p in bvap.ap]),
            )
            rep_bo = const.tile([P, D], F32)
            boap = bo_d[0, :]
            nc.gpsimd.dma_start(
                out=rep_bo,
                in_=bass.AP(tensor=boap.tensor, offset=boap.offset,
                            ap=[[0, P]] + [list(p) for p in boap.ap]),
            )

            # Causal masks for the 4 diagonal k-tile offsets, duplicated for
            # the 2 heads of a pair so the mask-mult uses a plain dense AP:
            # mask[p, v, hi, c] = 1.0 where (qb*QW + c) >= (j*P + p), v = j - 4*qb.
            cmask = const.tile([P, 4, 2, QW], BF16)
            nc.gpsimd.memset(cmask, 1.0)
            nc.gpsimd.affine_select(
                out=cmask, in_=cmask,
                pattern=[[-P, 4], [0, 2], [1, QW]],
                compare_op=mybir.AluOpType.is_ge,
                fill=0.0,
                base=0,
                channel_multiplier=-1,
            )


            # ---- resident tensors ----
            xT = res.tile([P, ND, S], BF16)      # x transposed [d, t]
            qT = res.tile([P, 2, S], BF16)       # Q^T [hk, t]
            kT = res.tile([P, 2, S], BF16)       # K^T [hk, t]
            v_sb = res.tile([P, NT, HPC, DH + 1], BF16)  # V (token-major), col 64 = 1.0
            zT = res.tile([P, 2, S], BF16)       # Z^T [hk, t], post-normalize

            nc.vector.memset(v_sb[:, :, :, DH : DH + 1], 1.0)

            dbg.update(xT=xT, qT=qT, kT=kT, v_sb=v_sb, zT=zT, cmask=cmask,
                       rep_bv=rep_bv, rep_bo=rep_bo, wq_sb=wq_sb, wo_sb=wo_sb,
                       bq_sb=bq_sb, ident=ident)

            # ---- stage A: load x (cast bf16), transpose, V-projection ----
            for tt in range(NT):
                xf = xstage.tile([P, D], F32, tag="xf")
                nc.sync.dma_start(out=xf, in_=x_d[tt * P : (tt + 1) * P, :])
                xb = xstage.tile([P, D], BF16, tag="xb")
                nc.vector.tensor_copy(out=xb, in_=xf)
                for dg in range(2):  # 4 transposes share one psum tile/drain
                    tp = zpool.tile([P, 4, P], BF16, tag="zo")
                    for di in range(4):
                        dj = dg * 4 + di
                        nc.tensor.transpose(
                            tp[:, di, :], xb[:, dj * P : (dj + 1) * P], ident
                        )
                    nc.vector.tensor_copy(
                        out=xT[:, dg * 4 : (dg + 1) * 4, tt * P : (tt + 1) * P],
                        in_=tp,
                    )
                ps = spool.tile([P, HK], F32, tag="s", name="vps")
                for dj in range(ND):
                    nc.tensor.matmul(
                        ps,
                        lhsT=xT[:, dj, tt * P : (tt + 1) * P],
                        rhs=wv_sb[:, dj, :],
                        start=(dj == 0),
                        stop=(dj == ND - 1),
                    )
                nc.vector.tensor_add(
                    v_sb[:, tt, :, 0:DH],
                    ps.rearrange("p (h w) -> p h w", h=HPC),
                    rep_bv.rearrange("p (h w) -> p h w", h=HPC),
                )

            # ---- stage B: Q/K projections (qb-major so attention starts early) ----
            for qb in range(NQB):
                for w_sb, b_sb, dst in ((wq_sb, bq_sb, qT), (wk_sb, bk_sb, kT)):
                    for mt in range(2):
                        ps = spool.tile([P, QW], F32, tag="s")
                        for dj in range(ND):
                            nc.tensor.matmul(
                                ps,
                                lhsT=w_sb[:, dj, mt * P : (mt + 1) * P],
                                rhs=xT[:, dj, qb * QW : (qb + 1) * QW],
                                start=(dj == 0),
                                stop=(dj == ND - 1),
                            )
                        nc.vector.tensor_scalar(
                            dst[:, mt, qb * QW : (qb + 1) * QW],
                            ps,
                            b_sb[:, mt : mt + 1],
                            None,
                            mybir.AluOpType.add,
                        )

            # ---- stage C: attention (head pairs for PE row-group packing) ----
            def issue_scores(m, qb, j):
                sp = spool.tile([P, 2, QW], F32, tag="s", name=f"sp_{m}_{qb}_{j}")
                for hi in range(2):
                    pb = hi * 64
                    nc.tensor.matmul(
                        sp[:, hi, :],
                        lhsT=kT[pb : pb + 64, m, j * P : (j + 1) * P],
                        rhs=qT[pb : pb + 64, m, qb * QW : (qb + 1) * QW],
                        start=True,
                        stop=True,
                    )
                return sp

            def emit_o_tile(tt, dc):
                op = zpool.tile([P, QW], F32, tag="zo", name="op")
                for kt2 in range(2):
                    nc.tensor.matmul(
                        op,
                        lhsT=zT[:, kt2, tt * P : (tt + 1) * P],
                        rhs=wo_sb[:, kt2, dc * QW : (dc + 1) * QW],
                        start=(kt2 == 0),
                        stop=(kt2 == 1),
                    )
                ost = drains.tile([P, QW], F32, tag="ost")
                nc.vector.tensor_add(ost, op, rep_bo[:, dc * QW : (dc + 1) * QW])
                nc.sync.dma_start(
                    out=out_d[tt * P : (tt + 1) * P, dc * QW : (dc + 1) * QW],
                    in_=ost,
                )

            o_pending = []
            for m in range(2):
                for qb in range(NQB):
                    nkt = 4 * qb + 4
                    zps = [
                        zpool.tile([DH + 1, QW], F32, tag="zo", name=f"zp{i}")
                        for i in range(2)
                    ]
                    sp_cur = issue_scores(m, qb, 0)
                    for j in range(nkt):
                        sp_next = issue_scores(m, qb, j + 1) if j + 1 < nkt else None
                        pt = pbuf.tile([P, 2, QW], BF16, tag="p")
                        nc.scalar.activation(
                            out=pt, in_=sp_cur,
                            func=mybir.ActivationFunctionType.Exp,
                            scale=SCALE,
                        )
                        if j >= 4 * qb:  # diagonal: zero where k > q
                            nc.vector.tensor_mul(pt, pt, cmask[:, j - 4 * qb, :, :])
                        for hi in range(2):
                            h = 2 * m + hi
                            nc.tensor.matmul(
                                zps[hi],
                                lhsT=v_sb[:, j, h, 0 : DH + 1],
                                rhs=pt[:, hi, :],
                                start=(j == 0),
                                stop=(j == nkt - 1),
                            )
                        if o_pending and j < 8:
                            emit_o_tile(*o_pending.pop(0))
                        sp_cur = sp_next
                    for hi in range(2):
                        ell = small.tile([DH + 1, QW], F32, tag="ell")
                        nc.vector.reciprocal_approx_fast(
                            out=ell[DH : DH + 1, :], in_=zps[hi][DH : DH + 1, :]
                        )
                        eld = dpool.tile([1, QW], F32, tag="eld")
                        nc.sync.dma_start(out=eld, in_=ell[DH : DH + 1, :])
                        rep = small.tile([DH, QW], F32, tag="rep")
                        e = eld[0, :]
                        nc.gpsimd.dma_start(
                            out=rep,
                            in_=bass.AP(tensor=e.tensor, offset=e.offset,
                                        ap=[[0, DH]] + [list(p) for p in e.ap]),
                        )
                        if hi == 0:
                            nc.vector.tensor_mul(
                                zT[0:DH, m, qb * QW : (qb + 1) * QW],
                                zps[hi][0:DH, :],
                                rep,
                            )
                        else:
                            zs = drains.tile([DH, QW], BF16, tag="zstage")
                            nc.vector.tensor_mul(zs, zps[hi][0:DH, :], rep)
                            nc.sync.dma_start(
                                out=zT[64:128, m, qb * QW : (qb + 1) * QW], in_=zs
                            )

                    # queue this q block's output projection; it is emitted
                    # interleaved into the next q block's attention (needs zT
                    # of both head pairs, so only after the m == 1 pass)
                    if m == 1:
                        o_pending = [
                            (tt, dc)
                            for tt in range(4 * qb, 4 * qb + 4)
                            for dc in range(2)
                        ]
            for tt, dc in o_pending:
                emit_o_tile(tt, dc)

            if hwdump:
                nc.sync.dma_start(
                    out=dump_d["dump_xT"][:, :].rearrange("p (a b) -> p a b", a=ND),
                    in_=xT)
                nc.sync.dma_start(
                    out=dump_d["dump_qT"][:, :].rearrange("p (a b) -> p a b", a=2),
                    in_=qT)
                nc.sync.dma_start(
                    out=dump_d["dump_kT"][:, :].rearrange("p (a b) -> p a b", a=2),
                    in_=kT)
                nc.sync.dma_start(
                    out=dump_d["dump_v"][:, :].rearrange(
                        "p (a b c) -> p a b c", a=NT, b=HPC),
                    in_=v_sb)
                nc.sync.dma_start(
                    out=dump_d["dump_zT"][:, :].rearrange("p (a b) -> p a b", a=2),
                    in_=zT)
                nc.sync.dma_start(
                    out=dump_d["dump_cmask"][:, :].rearrange("p (a b) -> p a b", a=4),
                    in_=cmask)

    nc.finalize()
    if debug:
        return nc, dbg
    return nc


_NC_CACHE = None


def get_nc():
    global _NC_CACHE
    if _NC_CACHE is None:
        _NC_CACHE = build_program()
    return _NC_CACHE


def shard_inputs(x, W_Q, W_K, W_V, W_O, b_Q, b_K, b_V, b_O):
    f = lambda a: np.ascontiguousarray(np.asarray(a), dtype=np.float32)
    in_maps = []
    for core in range(8):
        b, g = divmod(core, 4)
        hs = slice(g * HPC, (g + 1) * HPC)
        in_maps.append({
            "x": f(np.asarray(x)[b]),
            "wq": f(np.asarray(W_Q)[hs].transpose(1, 0, 2).reshape(D, HK)),
            "wk": f(np.asarray(W_K)[hs].transpose(1, 0, 2).reshape(D, HK)),
            "wv": f(np.asarray(W_V)[hs].transpose(1, 0, 2).reshape(D, HK)),
            "wo": f(np.asarray(W_O)[hs].reshape(HK, D)),
            "bq": f(np.asarray(b_Q)[hs].reshape(1, HK)),
            "bk": f(np.asarray(b_K)[hs].reshape(1, HK)),
            "bv": f(np.asarray(b_V)[hs].reshape(1, HK)),
            "bo": f(np.asarray(b_O) if g == 0 else np.zeros_like(np.asarray(b_O))).reshape(1, D),
        })
    return in_maps


def combine_outputs(results):
    out = np.zeros((B, S, D), dtype=np.float32)
    for core in range(8):
        b = core // 4
        out[b] += results[core]["out"]
    return out


def kernel(**inputs):
    nc = get_nc()
    in_maps = shard_inputs(**inputs)
    res = run_bass_kernel_spmd(nc, in_maps, list(range(8)))
    return combine_outputs(res.results)
